# revision 47
# baseline (speedup 1.0000x reference)
"""BertAttention (B=32, S=512, H=768, 12 heads) Bass/Tile kernel for 8 TRN2 cores.

Sharding: data-parallel over batch -- 4 batches per NeuronCore. kernel() takes
the FULL inputs, preps/slices on host, runs one SPMD NEFF on cores 0-7, and
reassembles the full (32, 512, 768) output.

All matmuls run as fp8e4 DoubleRow (2 contraction tiles per instruction at
0.5 cycles/row). Per batch:
  QKV projections: x(fp8) @ W.T(fp8, x64 host-scaled), contraction 768 = 3
    DoubleRow instructions; V bias enters as a K=1 ones-row matmul; Q bias and
    the 1/64 descale fold into the PSUM eviction; K bias is dropped entirely
    (softmax-invariant: (q+bq).(k+bk) ~ (q+bq).k modulo per-query constants).
  scores[k,q] = K^T Q per (pair, key-tile, head): contraction d=64 expressed
    as a DoubleRow pair (real tile, zero tile) so each [128,512] score block
    costs 256 PE cycles.
  exp: chunks of [128,1024] (both heads of a pair) computed as
    fp8(exp(s/8 - 2)) -- the -2 shift is softmax-invariant and keeps the
    Schraudolph chunks inside fp8's normal range. Chunks are split across the
    ACT engine (native Exp) and DVE/Pool (Schraudolph: uint8 bits =
    round(s*11.54/8 + offset), bitcast to fp8e4).
  attn.V: wev[0:64, hh*512:] = V'^T E per head (V' = (v+bv)*e^mask, fp8);
    rows 64:128 of the same PSUM tile get a broadcast softmax denominator via
    a matmul whose lhsT is the e^mask/8 column replicated 64x (zero-stride).
    Normalize: DVE reciprocal of the broadcast rows -> SBUF, then per-head
    multiplies straight out of PSUM into fp8 wt (= 8*attn weights).
  O-projection: wt^T Wo(x64) accumulating in PSUM, plus an identity-lhsT bf16
    matmul that adds the residual (host ships xres*512 in bf16; LayerNorm is
    scale-invariant so the 512x scaling drops out exactly). bn_stats/bn_aggr
    give mean/var; rstd = exp(-0.5 ln(var+eps')); normalize on ACT via
    per-partition scale/bias; bf16 output staged and DMA'd once per batch.

Host folds bo (and the 512x) into xres, ln_w/ln_b onto the output.
"""

import sys

for _p in ("/opt/trn_rl_repo",):
    if _p not in sys.path:
        sys.path.insert(0, _p)

import numpy as np
import ml_dtypes

BF16 = ml_dtypes.bfloat16
FP8 = ml_dtypes.float8_e4m3

N_CORES = 8
B_LOC = 4            # batches per core
S = 512              # sequence length
T = B_LOC * S        # tokens per core
H = 768              # hidden
NH = 12              # heads
D = 64               # head size
KT = 6               # 128-row hidden tiles
PAIRS = NH // 2      # head pairs == hidden j-tiles (6)
KT4 = S // 128       # 128-wide key/query token tiles per batch (4)

WSCALE = 64.0        # host premultiplier on all weight matrices
LOG2E8 = 11.5415603  # 8*log2(e): Schraudolph slope for exp(s/8)
ESHIFT = 2.0         # exp(s/8 - ESHIFT); softmax-invariant range shift
U8OFF = 56.0 - ESHIFT * LOG2E8  # Schraudolph offset (fp8e4 bias 7 -> 56)

# exp-chunk engine per (pr, kt): 'A' = ACT native, 'D' = DVE Schraudolph,
# 'P' = Pool Schraudolph.  17 A / 3 D / 4 P per batch.
EXP_ENGINE = {
    (0, 0): 'D', (0, 1): 'A', (0, 2): 'A', (0, 3): 'A',
    (1, 0): 'D', (1, 1): 'D', (1, 2): 'A', (1, 3): 'A',
    (2, 0): 'D', (2, 1): 'D', (2, 2): 'A', (2, 3): 'A',
    (3, 0): 'D', (3, 1): 'D', (3, 2): 'A', (3, 3): 'A',
    (4, 0): 'D', (4, 1): 'A', (4, 2): 'A', (4, 3): 'A',
    (5, 0): 'D', (5, 1): 'A', (5, 2): 'A', (5, 3): 'A',
}

_CACHE = {}


def _build():
    import concourse.bacc as bacc
    import concourse.tile as tile
    from concourse import mybir
    import concourse.bass as bass

    f32 = mybir.dt.float32
    bf16 = mybir.dt.bfloat16
    fp8 = mybir.dt.float8e4
    u8 = mybir.dt.uint8
    AF = mybir.ActivationFunctionType
    OP = mybir.AluOpType
    DR = mybir.MatmulPerfMode.DoubleRow

    nc = bacc.Bacc("TRN2", target_bir_lowering=False, debug=False,
                   enable_asserts=False, num_devices=N_CORES)

    xT_d = nc.dram_tensor("xT", [128, KT, T], fp8, kind="ExternalInput").ap()
    xres_d = nc.dram_tensor("xres", [T, H], bf16, kind="ExternalInput").ap()
    wq_d = nc.dram_tensor("wq", [128, KT, H], fp8, kind="ExternalInput").ap()
    wk_d = nc.dram_tensor("wk", [128, KT, H], fp8, kind="ExternalInput").ap()
    wv_d = nc.dram_tensor("wv", [128, KT, H], fp8, kind="ExternalInput").ap()
    wo_d = nc.dram_tensor("wo", [128, KT, H], fp8, kind="ExternalInput").ap()
    bq_d = nc.dram_tensor("bq", [128, KT], f32, kind="ExternalInput").ap()
    bvr_d = nc.dram_tensor("bvr", [1, H], fp8, kind="ExternalInput").ap()
    em64_d = nc.dram_tensor("em64", [128, B_LOC, KT4], f32, kind="ExternalInput").ap()
    em8r_d = nc.dram_tensor("em8r", [128, B_LOC, KT4, PAIRS, 64], fp8, kind="ExternalInput").ap()
    ident_d = nc.dram_tensor("ident", [128, 128], bf16, kind="ExternalInput").ap()
    out_d = nc.dram_tensor("out", [T, H], bf16, kind="ExternalOutput").ap()

    xres_t = xres_d.rearrange("(b tt p) h -> b p tt h", p=128, tt=KT4)
    out_t = out_d.rearrange("(b tt p) h -> b p tt h", p=128, tt=KT4)

    def dr_qk(t, hh, pr, off, n):
        """Zero-padded DoubleRow operand for the scores matmul: [64, 2, n]
        where tile 0 = data cols [off, off+n) and tile 1 = the zeros half at
        [512+off, 512+off+n)."""
        v = t[hh * 64:(hh + 1) * 64, pr, :]
        return bass.AP(tensor=v.tensor, offset=v.offset + off,
                       ap=[v.ap[0], [512, 2], [1, n]])

    with tile.TileContext(nc) as tc:
        with tc.tile_pool(name="persist", bufs=1) as persist, \
             tc.tile_pool(name="qk", bufs=2) as qkp, \
             tc.tile_pool(name="vbp", bufs=2) as vbp, \
             tc.tile_pool(name="exp", bufs=3) as exp_pool, \
             tc.tile_pool(name="wtp", bufs=3) as wtp, \
             tc.tile_pool(name="rsp", bufs=3) as rsp, \
             tc.tile_pool(name="xrp", bufs=2) as xrp, \
             tc.tile_pool(name="stp", bufs=3) as stp, \
             tc.tile_pool(name="smp", bufs=3) as smp, \
             tc.tile_pool(name="pp_ps", bufs=2, space="PSUM") as pp, \
             tc.tile_pool(name="sc_ps", bufs=2, space="PSUM") as scp, \
             tc.tile_pool(name="wev_ps", bufs=1, space="PSUM") as wvp:
            # ---- persistent tensors ----
            xT_sb = persist.tile([128, KT, T], fp8)
            wq_sb = persist.tile([128, KT, H], fp8)
            wk_sb = persist.tile([128, KT, H], fp8)
            wv_sb = persist.tile([128, KT, H], fp8)
            wo_sb = persist.tile([128, KT, H], fp8)
            bq_sb = persist.tile([128, KT], f32)
            bvr_sb = persist.tile([1, H], fp8)
            em64_sb = persist.tile([128, B_LOC, KT4], f32)
            ident_sb = persist.tile([128, 128], bf16)
            ones_sb = persist.tile([1, 128], fp8)
            eps_sb = persist.tile([128, 1], f32)
            shift_sb = persist.tile([128, 1], f32)

            nc.sync.dma_start(out=wq_sb, in_=wq_d)
            nc.sync.dma_start(out=xT_sb[:, :, 0:S], in_=xT_d[:, :, 0:S])
            nc.sync.dma_start(out=wk_sb, in_=wk_d)
            nc.sync.dma_start(out=wv_sb, in_=wv_d)
            nc.sync.dma_start(out=bq_sb, in_=bq_d)
            nc.sync.dma_start(out=bvr_sb, in_=bvr_d)
            nc.sync.dma_start(out=em64_sb, in_=em64_d)
            nc.vector.memset(ones_sb, 1.0)
            nc.vector.memset(eps_sb, 512.0 * 512.0 * 1e-12)
            nc.vector.memset(shift_sb, -ESHIFT)
            # Pre-load the one ACT LUT set this kernel ever needs (Exp, Ln,
            # Identity all live in natural_log_exp_and_others).
            _tables = list(__import__("concourse.hw_specs", fromlist=["x"])
                           .get_activation_tables(nc.m.arch))
            _set6 = _tables.index("natural_log_exp_and_others")
            nc.scalar.add_instruction(mybir.InstLoadActFuncSet(
                name=nc.get_next_instruction_name(), ins=[], outs=[],
                act_func_set_id=_set6))

            # ---- per-batch tile allocation + projection emission ----
            def alloc_proj(b):
                qb = qkp.tile([128, PAIRS, 1024], fp8, tag="qb")
                kb = qkp.tile([128, PAIRS, 1024], fp8, tag="kb")
                vb = vbp.tile([128, KT4, PAIRS, 384], fp8, tag="vb")
                if b < 2:  # zero blocks persist in the 2 physical ring slots
                    nc.gpsimd.memset(qb[:, :, 512:1024], 0.0)
                    nc.gpsimd.memset(kb[:, :, 512:1024], 0.0)
                    nc.gpsimd.memset(vb[:, :, :, 64:128], 0.0)
                    nc.gpsimd.memset(vb[:, :, :, 256:320], 0.0)
                # e^mask/8 blocks (cols 192:256 and 320:384), replicated 64x,
                # via one DMA (zero-stride block repeat) so no compute engine
                # pays for them
                e = em8r_d[:, b]
                for c0 in (192, 320):
                    nc.sync.dma_start(out=vb[:, :, :, c0:c0 + 64], in_=e)
                return qb, kb, vb

            def emit_qk_proj(b, pr, w_sb, dst, bias):
                ps = pp.tile([128, 512], f32, tag="proj")
                for ktp in range(0, KT, 2):
                    nc.tensor.matmul(
                        ps, w_sb[:, ktp:ktp + 2, pr * 128:(pr + 1) * 128],
                        xT_sb[:, ktp:ktp + 2, b * S:(b + 1) * S],
                        start=(ktp == 0), stop=(ktp == KT - 2), perf_mode=DR)
                nc.scalar.activation(dst[:, pr, 0:512], ps, AF.Identity,
                                     bias=(bias if bias is not None else 0.0),
                                     scale=1.0 / WSCALE)

            V_GROUPS = [(tl, c) for tl in range(KT4) for c in (0, 512)]
            V_SLICE = {0: [0], 1: [1], 2: [2], 3: [3], 4: [4, 5], 5: [6, 7]}

            def emit_v_group(b, vb, g):
                tl, c0 = V_GROUPS[g]
                n = 512 if c0 == 0 else H - c0
                ps = pp.tile([128, 512], f32, tag="proj")
                tt = b * KT4 + tl
                for ktp in range(0, KT, 2):
                    nc.tensor.matmul(
                        ps[:, 0:n], xT_sb[:, ktp:ktp + 2, tt * 128:(tt + 1) * 128],
                        wv_sb[:, ktp:ktp + 2, c0:c0 + n],
                        start=(ktp == 0), stop=False, perf_mode=DR)
                nc.tensor.matmul(ps[:, 0:n], ones_sb, bvr_sb[:, c0:c0 + n],
                                 start=False, stop=True)
                plo, phi = c0 // 128, (c0 + n) // 128
                dst = vb[:, tl, plo:phi, :].rearrange(
                    "p a (b c) -> p a b c", b=3, c=128)[:, :, 0:2, 0:64]
                nc.vector.tensor_scalar_mul(
                    dst, ps[:, 0:n].rearrange("p (a b d) -> p a b d", b=2, d=64),
                    em64_sb[:, b, tl:tl + 1])

            def emit_proj_slice(b, pr, tiles, vs=None):
                qb, kb, vb = tiles
                emit_qk_proj(b, pr, wq_sb, qb, bq_sb[:, pr:pr + 1])
                emit_qk_proj(b, pr, wk_sb, kb, None)
                for g in (V_SLICE[pr] if vs is None else vs):
                    emit_v_group(b, vb, g)

            def emit_o_ln(b, wt_sb, xr, stage, qt):
                """O-projection + identity-residual + LN for one query tile."""
                ogs = []
                for c0, n in ((0, 512), (512, 256)):
                    og = pp.tile([128, 512], f32, tag="proj")
                    for jtp in range(0, KT, 2):
                        nc.tensor.matmul(
                            og[:, 0:n],
                            wt_sb[:, jtp:jtp + 2, qt * 128:(qt + 1) * 128],
                            wo_sb[:, jtp:jtp + 2, c0:c0 + n],
                            start=(jtp == 0), stop=False, perf_mode=DR)
                    nc.tensor.matmul(og[:, 0:n], ident_sb,
                                     xr[:, qt, c0:c0 + n], start=False, stop=True)
                    ogs.append(og)
                yb = smp.tile([128, H], bf16, tag="yb")
                for (c0, n), og in zip(((0, 512), (512, 256)), ogs):
                    nc.scalar.activation(yb[:, c0:c0 + n], og[:, 0:n],
                                         AF.Identity, bias=0.0, scale=1.0)
                stats = smp.tile([128, 2, 6], f32, tag="st")
                nc.vector.bn_stats(stats[:, 0, :], yb[:, 0:512])
                nc.vector.bn_stats(stats[:, 1, :], yb[:, 512:768])
                mv = smp.tile([128, 2], f32, tag="mv")
                nc.vector.bn_aggr(mv, stats)
                lnv = smp.tile([128, 1], f32, tag="lnv")
                nc.scalar.activation(lnv, mv[:, 1:2], AF.Ln, bias=eps_sb, scale=1.0)
                rstd = smp.tile([128, 1], f32, tag="rstd")
                nc.scalar.activation(rstd, lnv, AF.Exp, bias=0.0, scale=-0.5)
                neng = nc.vector if b == B_LOC - 1 else nc.gpsimd
                neng.tensor_scalar(stage[:, qt, :], yb, scalar1=mv[:, 0:1],
                                   scalar2=rstd, op0=OP.subtract, op1=OP.mult)

            # ---- software pipeline over batches ----
            cur = alloc_proj(0)
            emit_proj_slice(0, 0, cur, vs=[0, 2])
            nc.sync.dma_start(out=ident_sb, in_=ident_d)
            for bb in range(1, B_LOC):
                nc.sync.dma_start(out=xT_sb[:, :, bb * S:(bb + 1) * S],
                                  in_=xT_d[:, :, bb * S:(bb + 1) * S])
            nc.sync.dma_start(out=wo_sb, in_=wo_d)

            prev_ln = None   # (wt_sb, xr, stage, b) of the previous batch
            pending_mm = None    # attnV+normalize delayed one pair (PE HOL)
            for b in range(B_LOC):
                qb, kb, vb = cur
                nxt = alloc_proj(b + 1) if b + 1 < B_LOC else None
                xr = xrp.tile([128, KT4, H], bf16, tag="xr")
                nc.sync.dma_start(out=xr, in_=xres_t[b])
                wt_sb = wtp.tile([128, KT, 512], fp8, tag="wt")

                for pr in range(PAIRS):
                    ex_t = exp_pool.tile([128, KT4, 2, 512], fp8, tag="ex")
                    # scores + exp per key tile
                    for kt in range(KT4):
                        sc = scp.tile([128, 1024], f32, tag="sc")
                        for hh in range(2):
                            nc.tensor.matmul(
                                sc[:, hh * 512:(hh + 1) * 512],
                                dr_qk(kb, hh, pr, kt * 128, 128),
                                dr_qk(qb, hh, pr, 0, 512),
                                start=True, stop=True, perf_mode=DR)
                        eng = EXP_ENGINE[(pr, kt)]
                        if b == B_LOC - 1 and pr >= 4:
                            eng = 'A'
                        if eng == 'A':
                            nc.scalar.activation(
                                ex_t[:, kt, :, :].rearrange("p a q -> p (a q)"),
                                sc, AF.Exp, bias=shift_sb, scale=0.125)
                        else:
                            veng = nc.vector if eng == 'D' else nc.gpsimd
                            veng.tensor_scalar(
                                ex_t[:, kt, :, :].rearrange(
                                    "p a q -> p (a q)").bitcast(u8),
                                sc, scalar1=LOG2E8 / 8.0, scalar2=U8OFF,
                                op0=OP.mult, op1=OP.add)
                    if b == 0 and pr < PAIRS - 1:
                        B0_VS = {0: [4, 6], 1: [1], 2: [3], 3: [5], 4: [7], 5: []}
                        emit_proj_slice(0, pr + 1, cur, vs=B0_VS[pr])
                    if nxt is not None:
                        emit_proj_slice(b + 1, pr, nxt)
                    if prev_ln is not None and 1 <= pr <= 4:
                        emit_o_ln(prev_ln[3], prev_ln[0], prev_ln[1],
                                  prev_ln[2], pr - 1)
                        if pr == 4:
                            nc.sync.dma_start(out=out_t[prev_ln[3]],
                                              in_=prev_ln[2])
                            prev_ln = None
                    def make_attn(pr, ex_t, vb, wt_sb):
                        def emit_attn():
                            # attn.V: hh0/hh1 stacked at rows 0:64 / 64:128 by
                            # zero-padded lhsT halves in one PSUM group; cols
                            # 512:1024 get the broadcast softmax denominator
                            # (em8-replicated lhsT blocks)
                            wev = wvp.tile([128, 1024], f32, tag="wev")
                            for hh in range(2):
                                for ktp in range(0, KT4, 2):
                                    nc.tensor.matmul(
                                        wev[:, 512:1024],
                                        vb[:, ktp:ktp + 2, pr, 192 + 64 * hh:320 + 64 * hh],
                                        ex_t[:, ktp:ktp + 2, hh, :],
                                        start=(hh == 0 and ktp == 0),
                                        stop=(hh == 1 and ktp == 2), perf_mode=DR)
                            for hh in range(2):
                                for ktp in range(0, KT4, 2):
                                    nc.tensor.matmul(
                                        wev[:, 0:512],
                                        vb[:, ktp:ktp + 2, pr, 64 * hh:64 * hh + 128],
                                        ex_t[:, ktp:ktp + 2, hh, :],
                                        start=(hh == 0 and ktp == 0),
                                        stop=(hh == 1 and ktp == 2), perf_mode=DR)
                            rs = rsp.tile([128, 512], f32, tag="rs")
                            nc.vector.reciprocal(rs, wev[:, 512:1024])
                            nc.vector.tensor_tensor(out=wt_sb[:, pr, :],
                                                    in0=wev[:, 0:512], in1=rs,
                                                    op=OP.mult)
                        return emit_attn

                    if pending_mm is not None:
                        pending_mm()
                        pending_mm = None
                    this_attn = make_attn(pr, ex_t, vb, wt_sb)
                    if b == B_LOC - 1:
                        this_attn()
                    else:
                        pending_mm = this_attn

                stage = stp.tile([128, KT4, H], bf16, tag="stage")
                if b == B_LOC - 1:
                    for qt in range(KT4):
                        emit_o_ln(b, wt_sb, xr, stage, qt)
                        nc.sync.dma_start(out=out_t[b][:, qt], in_=stage[:, qt])
                else:
                    prev_ln = (wt_sb, xr, stage, b)
                cur = nxt

    nc.compile()
    return nc


def _get_nc():
    if "nc" not in _CACHE:
        _CACHE["nc"] = _build()
    return _CACHE["nc"]


def _prep_in_maps(inputs):
    x = np.asarray(inputs["x"], np.float32)
    mask = np.asarray(inputs["additive_attention_mask"], np.float32)[:, 0, 0, :]
    bo = np.asarray(inputs["bo"], np.float32)

    def wprep(w):
        wt = np.asarray(w, np.float32).T * WSCALE        # [in, out]
        return np.ascontiguousarray(
            wt.reshape(KT, 128, H).transpose(1, 0, 2)).astype(FP8)

    shared = {
        "wq": wprep(inputs["Wq"]),
        "wk": wprep(inputs["Wk"]),
        "wv": wprep(inputs["Wv"]),
        "wo": wprep(inputs["Wo"]),
        "bq": np.ascontiguousarray(
            np.asarray(inputs["bq"], np.float32).reshape(KT, 128).T),
        "bvr": np.ascontiguousarray(
            (np.asarray(inputs["bv"], np.float32) * WSCALE)[None, :]).astype(FP8),
        "ident": np.eye(128, dtype=np.float32).astype(BF16),
    }
    in_maps = []
    for c in range(N_CORES):
        xs = x[c * B_LOC:(c + 1) * B_LOC].reshape(T, H)
        mk = mask[c * B_LOC:(c + 1) * B_LOC]             # [B_LOC, S]
        em = np.exp(mk).reshape(B_LOC, KT4, 128)
        em64 = np.ascontiguousarray(em.transpose(2, 0, 1) / WSCALE)
        em8r = np.repeat(np.repeat(
            (em.transpose(2, 0, 1) / 8.0)[:, :, :, None, None], PAIRS, axis=3),
            64, axis=4)
        in_maps.append({
            "xT": np.ascontiguousarray(
                xs.T.reshape(KT, 128, T).transpose(1, 0, 2)).astype(FP8),
            "xres": np.ascontiguousarray((xs + bo[None, :]) * 512.0).astype(BF16),
            "em64": em64.astype(np.float32),
            "em8r": np.ascontiguousarray(em8r).astype(FP8),
            **shared,
        })
    return in_maps


def run(inputs, trace=False):
    """Returns (full_output, BassKernelResults)."""
    from concourse.bass_utils import run_bass_kernel_spmd

    nc = _get_nc()
    in_maps = _prep_in_maps(inputs)
    res = run_bass_kernel_spmd(nc, in_maps, core_ids=list(range(N_CORES)),
                               trace=trace)
    out = np.concatenate(
        [res.results[c]["out"].astype(np.float32).reshape(B_LOC, S, H)
         for c in range(N_CORES)], axis=0)
    ln_w = np.asarray(inputs["ln_w"], np.float32)
    ln_b = np.asarray(inputs["ln_b"], np.float32)
    out = out * ln_w[None, None, :] + ln_b[None, None, :]
    return np.ascontiguousarray(out.astype(np.float32)), res


def kernel(**inputs) -> np.ndarray:
    out, _ = run(inputs, trace=False)
    return out
